# revision 1
# baseline (speedup 1.0000x reference)
"""2-layer GCN (DGL GraphConv, norm='both') on 8 trn2 NeuronCores.

Strategy:
  - Fold both GraphConv norms into per-edge weights cw[e] = outdeg(src[e])^-1/2 * indeg(dst[e])^-1/2.
    Then layer1 = relu((A_cw @ x) @ W1 + b1), layer2 = (A_cw @ h) @ W2 + b2 (W right-mult
    commutes with row-space aggregation).
  - Shard destination nodes across 8 cores (12544 padded rows each, 98 tiles of 128).
  - Edges sorted/bucketed by dst tile on host, padded to a uniform number of
    128-edge blocks per tile. Aggregation per block: PSUM += msgs.T @ onehot(dst_local)
    on the tensor engine; messages gathered by row via indirect DMA.
  - x is replicated; between layers, z = h @ W2 shards are AllGathered so layer-2
    gathers can read any source node's row.
"""
import sys
sys.path.insert(0, "/opt/trn_rl_repo")
import numpy as np

NCORES = 8
P = 128

LAST_RESULT = None  # for test.py profiling introspection


def _build_edge_buckets(src, dst, cw, n_pad):
    """Sort edges by dst tile, pad each tile's edge list to a uniform block count.

    Returns (esrc, edstl, ecw) each shaped [n_tiles*NBLK, 128] (block-major),
    plus NBLK."""
    n_tiles = n_pad // P
    tile_id = dst // P
    order = np.argsort(tile_id, kind="stable")
    src_s, dst_s, cw_s = src[order], dst[order], cw[order]
    cnt = np.bincount(tile_id, minlength=n_tiles)
    nblk = int(np.ceil(cnt.max() / P))
    slots_per_tile = nblk * P
    total = n_tiles * slots_per_tile
    # position of each sorted edge inside its tile bucket
    starts = np.zeros(n_tiles + 1, np.int64)
    np.cumsum(cnt, out=starts[1:])
    pos_in_tile = np.arange(len(src)) - starts[tile_id[order]]
    slot = tile_id[order] * slots_per_tile + pos_in_tile
    esrc = np.zeros(total, np.int32)
    edstl = np.full(total, -1.0, np.float32)
    ecw = np.zeros(total, np.float32)
    esrc[slot] = src_s
    edstl[slot] = (dst_s - tile_id[order] * P).astype(np.float32)
    ecw[slot] = cw_s
    return esrc.reshape(-1, P), edstl.reshape(-1, P), ecw.reshape(-1, P), nblk


def _build_program(T, NBLK, NSH, NPAD, dt_tab, dt_f32):
    from concourse import bass, bacc, mybir, tile

    nc = bacc.Bacc(None, num_devices=NCORES)
    xb = nc.declare_dram_parameter("xb", [NPAD, P], dt_tab, isOutput=False)
    esrc = nc.declare_dram_parameter("esrc", [P, T * NBLK], mybir.dt.int32, isOutput=False)
    edstl = nc.declare_dram_parameter("edstl", [P, T * NBLK], dt_f32, isOutput=False)
    ecw = nc.declare_dram_parameter("ecw", [P, T * NBLK], dt_f32, isOutput=False)
    w1 = nc.declare_dram_parameter("w1", [P, P], dt_tab, isOutput=False)
    b1 = nc.declare_dram_parameter("b1", [P, 1], dt_f32, isOutput=False)
    w2 = nc.declare_dram_parameter("w2", [P, 64], dt_tab, isOutput=False)
    b2 = nc.declare_dram_parameter("b2", [P, 64], dt_f32, isOutput=False)
    iota = nc.declare_dram_parameter("iota", [P, P], dt_f32, isOutput=False)
    out = nc.declare_dram_parameter("out", [NSH, 64], mybir.dt.float32, isOutput=True)

    zsh = nc.dram_tensor("zsh", [NSH, 64], dt_tab, kind="Internal")
    zfull = nc.dram_tensor("zfull", [NPAD, 64], dt_tab, kind="Internal")

    TT = tile.TileContext

    # ---------------- layer 1 ----------------
    with TT(nc) as tc:
        with (
            tc.tile_pool(name="const", bufs=1) as cp,
            tc.tile_pool(name="sb", bufs=4) as sp,
            tc.tile_pool(name="ps", bufs=2, space="PSUM") as pp,
        ):
            w1t = cp.tile([P, P], dt_tab)
            nc.sync.dma_start(out=w1t[:], in_=w1[:])
            w2t = cp.tile([P, 64], dt_tab)
            nc.sync.dma_start(out=w2t[:], in_=w2[:])
            b1t = cp.tile([P, 1], dt_f32)
            nc.sync.dma_start(out=b1t[:], in_=b1[:])
            iot = cp.tile([P, P], dt_f32)
            nc.sync.dma_start(out=iot[:], in_=iota[:])
            esrc_t = cp.tile([P, T * NBLK], mybir.dt.int32)
            nc.sync.dma_start(out=esrc_t[:], in_=esrc[:])
            edstl_t = cp.tile([P, T * NBLK], dt_f32)
            nc.sync.dma_start(out=edstl_t[:], in_=edstl[:])
            ecw_t = cp.tile([P, T * NBLK], dt_f32)
            nc.sync.dma_start(out=ecw_t[:], in_=ecw[:])

            for t in range(T):
                psum_m = pp.tile([P, P], mybir.dt.float32, tag="pm")  # [in_f, n]
                for b in range(NBLK):
                    col = t * NBLK + b
                    msgs = sp.tile([P, P], dt_tab, tag="msgs")
                    nc.gpsimd.indirect_dma_start(
                        out=msgs[:], out_offset=None, in_=xb[:],
                        in_offset=bass.IndirectOffsetOnAxis(
                            ap=esrc_t[:, col:col + 1], axis=0),
                    )
                    msgs_w = sp.tile([P, P], dt_tab, tag="msgsw")
                    nc.vector.tensor_tensor(
                        out=msgs_w[:], in0=msgs[:],
                        in1=ecw_t[:, col:col + 1].to_broadcast([P, P]),
                        op=mybir.AluOpType.mult,
                    )
                    onehot = sp.tile([P, P], dt_tab, tag="oh")
                    nc.any.tensor_tensor(
                        out=onehot[:],
                        in0=edstl_t[:, col:col + 1].to_broadcast([P, P]),
                        in1=iot[:],
                        op=mybir.AluOpType.is_equal,
                    )
                    nc.tensor.matmul(
                        out=psum_m[:], lhsT=msgs_w[:], rhs=onehot[:],
                        start=(b == 0), stop=(b == NBLK - 1),
                    )
                mt = sp.tile([P, P], dt_tab, tag="mt")  # M.T = [in_f, n]
                nc.vector.tensor_copy(out=mt[:], in_=psum_m[:])
                psum_h = pp.tile([P, P], mybir.dt.float32, tag="ph")  # [out_f, n]
                nc.tensor.matmul(out=psum_h[:], lhsT=w1t[:], rhs=mt[:],
                                 start=True, stop=True)
                ht = sp.tile([P, P], dt_tab, tag="ht")  # [out_f, n]
                nc.scalar.activation(
                    out=ht[:], in_=psum_h[:],
                    func=mybir.ActivationFunctionType.Relu,
                    bias=b1t[:, :1], scale=1.0,
                )
                psum_z = pp.tile([P, 64], mybir.dt.float32, tag="pz")  # [n, 64]
                nc.tensor.matmul(out=psum_z[:], lhsT=ht[:], rhs=w2t[:],
                                 start=True, stop=True)
                zt = sp.tile([P, 64], dt_tab, tag="zt")
                nc.vector.tensor_copy(out=zt[:], in_=psum_z[:])
                nc.sync.dma_start(out=zsh[t * P:(t + 1) * P, :], in_=zt[:])

    # ---------------- allgather z ----------------
    with nc.semaphore("cc_sem") as cc_sem:
        nc.gpsimd.collective_compute(
            "AllGather", mybir.AluOpType.bypass,
            replica_groups=[list(range(NCORES))],
            ins=[zsh[:]], outs=[zfull[:]],
        ).then_inc(cc_sem, 1)
        nc.sync.wait_ge(cc_sem, 1)
        nc.all_engine_barrier()

    # ---------------- layer 2 ----------------
    with TT(nc) as tc:
        with (
            tc.tile_pool(name="const2", bufs=1) as cp2,
            tc.tile_pool(name="sb2", bufs=4) as sp2,
            tc.tile_pool(name="ps2", bufs=2, space="PSUM") as pp2,
        ):
            b2t = cp2.tile([P, 64], dt_f32)
            nc.sync.dma_start(out=b2t[:], in_=b2[:])
            iot2 = cp2.tile([P, P], dt_f32)
            nc.sync.dma_start(out=iot2[:], in_=iota[:])
            esrc2_t = cp2.tile([P, T * NBLK], mybir.dt.int32)
            nc.sync.dma_start(out=esrc2_t[:], in_=esrc[:])
            edstl2_t = cp2.tile([P, T * NBLK], dt_f32)
            nc.sync.dma_start(out=edstl2_t[:], in_=edstl[:])
            ecw2_t = cp2.tile([P, T * NBLK], dt_f32)
            nc.sync.dma_start(out=ecw2_t[:], in_=ecw[:])

            for t in range(T):
                psum_o = pp2.tile([P, 64], mybir.dt.float32, tag="po")  # [n, 64]
                for b in range(NBLK):
                    col = t * NBLK + b
                    msgs2 = sp2.tile([P, 64], dt_tab, tag="m2")
                    nc.gpsimd.indirect_dma_start(
                        out=msgs2[:], out_offset=None, in_=zfull[:],
                        in_offset=bass.IndirectOffsetOnAxis(
                            ap=esrc2_t[:, col:col + 1], axis=0),
                    )
                    msgs2_w = sp2.tile([P, 64], dt_tab, tag="m2w")
                    nc.vector.tensor_tensor(
                        out=msgs2_w[:], in0=msgs2[:],
                        in1=ecw2_t[:, col:col + 1].to_broadcast([P, 64]),
                        op=mybir.AluOpType.mult,
                    )
                    onehot2 = sp2.tile([P, P], dt_tab, tag="oh2")
                    nc.any.tensor_tensor(
                        out=onehot2[:],
                        in0=edstl2_t[:, col:col + 1].to_broadcast([P, P]),
                        in1=iot2[:],
                        op=mybir.AluOpType.is_equal,
                    )
                    nc.tensor.matmul(
                        out=psum_o[:], lhsT=onehot2[:], rhs=msgs2_w[:],
                        start=(b == 0), stop=(b == NBLK - 1),
                    )
                ot = sp2.tile([P, 64], mybir.dt.float32, tag="ot")
                nc.vector.tensor_tensor(out=ot[:], in0=psum_o[:], in1=b2t[:],
                                        op=mybir.AluOpType.add)
                nc.sync.dma_start(out=out[t * P:(t + 1) * P, :], in_=ot[:])

    nc.finalize()
    return nc


def kernel(in_feat, src, dst, W1, b1, W2, b2):
    global LAST_RESULT
    from concourse import mybir
    from concourse.bass_utils import run_bass_kernel_spmd

    in_feat = np.asarray(in_feat, np.float32)
    src = np.asarray(src, np.int32)
    dst = np.asarray(dst, np.int32)
    W1 = np.asarray(W1, np.float32)
    b1 = np.asarray(b1, np.float32)
    W2 = np.asarray(W2, np.float32)
    b2 = np.asarray(b2, np.float32)

    N, F = in_feat.shape          # 100000, 128
    H = W1.shape[1]               # 128
    O = W2.shape[1]               # 64
    assert F == P and H == P
    NPAD = int(np.ceil(N / (NCORES * P))) * NCORES * P   # 100352
    NSH = NPAD // NCORES                                  # 12544
    T = NSH // P                                          # 98

    deg_out = np.maximum(np.bincount(src, minlength=N), 1).astype(np.float32)
    deg_in = np.maximum(np.bincount(dst, minlength=N), 1).astype(np.float32)
    cw = (deg_out[src] ** -0.5) * (deg_in[dst] ** -0.5)

    esrc_b, edstl_b, ecw_b, NBLK = _build_edge_buckets(src, dst, cw, NPAD)
    # esrc_b: [n_tiles*NBLK, 128] block-major; per-core slice then -> [128, T*NBLK]

    xb = np.zeros((NPAD, P), np.float32)
    xb[:N] = in_feat
    iota_np = np.tile(np.arange(P, dtype=np.float32), (P, 1))
    b1c = b1.reshape(P, 1).astype(np.float32)
    b2c = np.tile(b2.reshape(1, O), (P, 1)).astype(np.float32)

    dt_tab = mybir.dt.float32
    dt_f32 = mybir.dt.float32
    nc = _build_program(T, NBLK, NSH, NPAD, dt_tab, dt_f32)

    in_maps = []
    for c in range(NCORES):
        lo, hi = c * T * NBLK, (c + 1) * T * NBLK
        in_maps.append({
            "xb": xb,
            "esrc": np.ascontiguousarray(esrc_b[lo:hi].T),
            "edstl": np.ascontiguousarray(edstl_b[lo:hi].T),
            "ecw": np.ascontiguousarray(ecw_b[lo:hi].T),
            "w1": W1,
            "b1": b1c,
            "w2": W2,
            "b2": b2c,
            "iota": iota_np,
        })

    res = run_bass_kernel_spmd(nc, in_maps, list(range(NCORES)))
    LAST_RESULT = res
    out_full = np.concatenate([res.results[c]["out"] for c in range(NCORES)], axis=0)
    return out_full[:N].astype(np.float32)


# revision 2
# speedup vs baseline: 1.0173x; 1.0173x over previous
"""2-layer GCN (DGL GraphConv, norm='both') on 8 trn2 NeuronCores.

Strategy:
  - Fold both GraphConv norms into per-edge weights cw[e] = outdeg(src[e])^-1/2 * indeg(dst[e])^-1/2.
    Then layer1 = relu((A_cw @ x) @ W1 + b1), layer2 = (A_cw @ h) @ W2 + b2 (W right-mult
    commutes with row-space aggregation).
  - Shard destination nodes across 8 cores (12544 padded rows each, 98 tiles of 128).
  - Edges sorted/bucketed by dst tile on host, padded to a uniform number of
    128-edge blocks per tile. Aggregation per block: PSUM += msgs.T @ onehot(dst_local)
    on the tensor engine; messages gathered by row via indirect DMA.
  - x is replicated; between layers, z = h @ W2 shards are AllGathered so layer-2
    gathers can read any source node's row.
"""
import sys
sys.path.insert(0, "/opt/trn_rl_repo")
import numpy as np

NCORES = 8
P = 128

LAST_RESULT = None  # for test.py profiling introspection


def _build_edge_buckets(src, dst, cw, n_pad):
    """Sort edges by dst tile, pad each tile's edge list to a uniform block count.

    Returns (esrc, edstl, ecw) each shaped [n_tiles*NBLK, 128] (block-major),
    plus NBLK."""
    n_tiles = n_pad // P
    tile_id = dst // P
    order = np.argsort(tile_id, kind="stable")
    src_s, dst_s, cw_s = src[order], dst[order], cw[order]
    cnt = np.bincount(tile_id, minlength=n_tiles)
    nblk = int(np.ceil(cnt.max() / P))
    slots_per_tile = nblk * P
    total = n_tiles * slots_per_tile
    # position of each sorted edge inside its tile bucket
    starts = np.zeros(n_tiles + 1, np.int64)
    np.cumsum(cnt, out=starts[1:])
    pos_in_tile = np.arange(len(src)) - starts[tile_id[order]]
    slot = tile_id[order] * slots_per_tile + pos_in_tile
    esrc = np.zeros(total, np.int32)
    edstl = np.full(total, -1.0, np.float32)
    ecw = np.zeros(total, np.float32)
    esrc[slot] = src_s
    edstl[slot] = (dst_s - tile_id[order] * P).astype(np.float32)
    ecw[slot] = cw_s
    return esrc.reshape(-1, P), edstl.reshape(-1, P), ecw.reshape(-1, P), nblk


def _build_program(T, NBLK, NSH, NPAD, dt_tab, dt_f32):
    from concourse import bass, bacc, mybir, tile

    nc = bacc.Bacc(None, num_devices=NCORES)
    xb = nc.declare_dram_parameter("xb", [NPAD, P], dt_tab, isOutput=False)
    esrc = nc.declare_dram_parameter("esrc", [P, T * NBLK], mybir.dt.int32, isOutput=False)
    edstl = nc.declare_dram_parameter("edstl", [P, T * NBLK], dt_f32, isOutput=False)
    ecw = nc.declare_dram_parameter("ecw", [P, T * NBLK], dt_f32, isOutput=False)
    w1 = nc.declare_dram_parameter("w1", [P, P], dt_tab, isOutput=False)
    b1 = nc.declare_dram_parameter("b1", [P, 1], mybir.dt.float32, isOutput=False)
    w2 = nc.declare_dram_parameter("w2", [P, 64], dt_tab, isOutput=False)
    b2 = nc.declare_dram_parameter("b2", [P, 64], mybir.dt.float32, isOutput=False)
    iota = nc.declare_dram_parameter("iota", [P, P], dt_f32, isOutput=False)
    out = nc.declare_dram_parameter("out", [NSH, 64], mybir.dt.float32, isOutput=True)

    zsh = nc.dram_tensor("zsh", [NSH, 64], dt_tab, kind="Internal")
    zfull = nc.dram_tensor("zfull", [NPAD, 64], dt_tab, kind="Internal")

    TT = tile.TileContext

    # ---------------- layer 1 ----------------
    with TT(nc) as tc:
        with (
            tc.tile_pool(name="const", bufs=1) as cp,
            tc.tile_pool(name="sb", bufs=4) as sp,
            tc.tile_pool(name="ps", bufs=2, space="PSUM") as pp,
        ):
            w1t = cp.tile([P, P], dt_tab)
            nc.sync.dma_start(out=w1t[:], in_=w1[:])
            w2t = cp.tile([P, 64], dt_tab)
            nc.sync.dma_start(out=w2t[:], in_=w2[:])
            b1t = cp.tile([P, 1], mybir.dt.float32)
            nc.sync.dma_start(out=b1t[:], in_=b1[:])
            iot = cp.tile([P, P], dt_f32)
            nc.sync.dma_start(out=iot[:], in_=iota[:])
            esrc_t = cp.tile([P, T * NBLK], mybir.dt.int32)
            nc.sync.dma_start(out=esrc_t[:], in_=esrc[:])
            edstl_t = cp.tile([P, T * NBLK], dt_f32)
            nc.sync.dma_start(out=edstl_t[:], in_=edstl[:])
            ecw_t = cp.tile([P, T * NBLK], dt_f32)
            nc.sync.dma_start(out=ecw_t[:], in_=ecw[:])

            for t in range(T):
                psum_m = pp.tile([P, P], mybir.dt.float32, tag="pm")  # [in_f, n]
                for b in range(NBLK):
                    col = t * NBLK + b
                    msgs = sp.tile([P, P], dt_tab, tag="msgs")
                    nc.gpsimd.indirect_dma_start(
                        out=msgs[:], out_offset=None, in_=xb[:],
                        in_offset=bass.IndirectOffsetOnAxis(
                            ap=esrc_t[:, col:col + 1], axis=0),
                    )
                    msgs_w = sp.tile([P, P], dt_tab, tag="msgsw")
                    nc.vector.tensor_tensor(
                        out=msgs_w[:], in0=msgs[:],
                        in1=ecw_t[:, col:col + 1].to_broadcast([P, P]),
                        op=mybir.AluOpType.mult,
                    )
                    onehot = sp.tile([P, P], dt_tab, tag="oh")
                    nc.any.tensor_tensor(
                        out=onehot[:],
                        in0=edstl_t[:, col:col + 1].to_broadcast([P, P]),
                        in1=iot[:],
                        op=mybir.AluOpType.is_equal,
                    )
                    nc.tensor.matmul(
                        out=psum_m[:], lhsT=msgs_w[:], rhs=onehot[:],
                        start=(b == 0), stop=(b == NBLK - 1),
                    )
                mt = sp.tile([P, P], dt_tab, tag="mt")  # M.T = [in_f, n]
                nc.vector.tensor_copy(out=mt[:], in_=psum_m[:])
                psum_h = pp.tile([P, P], mybir.dt.float32, tag="ph")  # [out_f, n]
                nc.tensor.matmul(out=psum_h[:], lhsT=w1t[:], rhs=mt[:],
                                 start=True, stop=True)
                ht = sp.tile([P, P], dt_tab, tag="ht")  # [out_f, n]
                nc.scalar.activation(
                    out=ht[:], in_=psum_h[:],
                    func=mybir.ActivationFunctionType.Relu,
                    bias=b1t[:, :1], scale=1.0,
                )
                psum_z = pp.tile([P, 64], mybir.dt.float32, tag="pz")  # [n, 64]
                nc.tensor.matmul(out=psum_z[:], lhsT=ht[:], rhs=w2t[:],
                                 start=True, stop=True)
                zt = sp.tile([P, 64], dt_tab, tag="zt")
                nc.vector.tensor_copy(out=zt[:], in_=psum_z[:])
                nc.sync.dma_start(out=zsh[t * P:(t + 1) * P, :], in_=zt[:])

    # ---------------- allgather z ----------------
    with nc.semaphore("cc_sem") as cc_sem:
        nc.gpsimd.collective_compute(
            "AllGather", mybir.AluOpType.bypass,
            replica_groups=[list(range(NCORES))],
            ins=[zsh[:]], outs=[zfull[:]],
        ).then_inc(cc_sem, 1)
        nc.sync.wait_ge(cc_sem, 1)
        nc.all_engine_barrier()

    # ---------------- layer 2 ----------------
    with TT(nc) as tc:
        with (
            tc.tile_pool(name="const2", bufs=1) as cp2,
            tc.tile_pool(name="sb2", bufs=4) as sp2,
            tc.tile_pool(name="ps2", bufs=2, space="PSUM") as pp2,
        ):
            b2t = cp2.tile([P, 64], mybir.dt.float32)
            nc.sync.dma_start(out=b2t[:], in_=b2[:])
            iot2 = cp2.tile([P, P], dt_f32)
            nc.sync.dma_start(out=iot2[:], in_=iota[:])
            esrc2_t = cp2.tile([P, T * NBLK], mybir.dt.int32)
            nc.sync.dma_start(out=esrc2_t[:], in_=esrc[:])
            edstl2_t = cp2.tile([P, T * NBLK], dt_f32)
            nc.sync.dma_start(out=edstl2_t[:], in_=edstl[:])
            ecw2_t = cp2.tile([P, T * NBLK], dt_f32)
            nc.sync.dma_start(out=ecw2_t[:], in_=ecw[:])

            for t in range(T):
                psum_o = pp2.tile([P, 64], mybir.dt.float32, tag="po")  # [n, 64]
                for b in range(NBLK):
                    col = t * NBLK + b
                    msgs2 = sp2.tile([P, 64], dt_tab, tag="m2")
                    nc.gpsimd.indirect_dma_start(
                        out=msgs2[:], out_offset=None, in_=zfull[:],
                        in_offset=bass.IndirectOffsetOnAxis(
                            ap=esrc2_t[:, col:col + 1], axis=0),
                    )
                    msgs2_w = sp2.tile([P, 64], dt_tab, tag="m2w")
                    nc.vector.tensor_tensor(
                        out=msgs2_w[:], in0=msgs2[:],
                        in1=ecw2_t[:, col:col + 1].to_broadcast([P, 64]),
                        op=mybir.AluOpType.mult,
                    )
                    onehot2 = sp2.tile([P, P], dt_tab, tag="oh2")
                    nc.any.tensor_tensor(
                        out=onehot2[:],
                        in0=edstl2_t[:, col:col + 1].to_broadcast([P, P]),
                        in1=iot2[:],
                        op=mybir.AluOpType.is_equal,
                    )
                    nc.tensor.matmul(
                        out=psum_o[:], lhsT=onehot2[:], rhs=msgs2_w[:],
                        start=(b == 0), stop=(b == NBLK - 1),
                    )
                ot = sp2.tile([P, 64], mybir.dt.float32, tag="ot")
                nc.vector.tensor_tensor(out=ot[:], in0=psum_o[:], in1=b2t[:],
                                        op=mybir.AluOpType.add)
                nc.sync.dma_start(out=out[t * P:(t + 1) * P, :], in_=ot[:])

    nc.finalize()
    return nc


def kernel(in_feat, src, dst, W1, b1, W2, b2):
    global LAST_RESULT
    from concourse import mybir
    from concourse.bass_utils import run_bass_kernel_spmd

    in_feat = np.asarray(in_feat, np.float32)
    src = np.asarray(src, np.int32)
    dst = np.asarray(dst, np.int32)
    W1 = np.asarray(W1, np.float32)
    b1 = np.asarray(b1, np.float32)
    W2 = np.asarray(W2, np.float32)
    b2 = np.asarray(b2, np.float32)

    N, F = in_feat.shape          # 100000, 128
    H = W1.shape[1]               # 128
    O = W2.shape[1]               # 64
    assert F == P and H == P
    NPAD = int(np.ceil(N / (NCORES * P))) * NCORES * P   # 100352
    NSH = NPAD // NCORES                                  # 12544
    T = NSH // P                                          # 98

    deg_out = np.maximum(np.bincount(src, minlength=N), 1).astype(np.float32)
    deg_in = np.maximum(np.bincount(dst, minlength=N), 1).astype(np.float32)
    cw = (deg_out[src] ** -0.5) * (deg_in[dst] ** -0.5)

    esrc_b, edstl_b, ecw_b, NBLK = _build_edge_buckets(src, dst, cw, NPAD)
    # esrc_b: [n_tiles*NBLK, 128] block-major; per-core slice then -> [128, T*NBLK]

    xb = np.zeros((NPAD, P), np.float32)
    xb[:N] = in_feat
    iota_np = np.tile(np.arange(P, dtype=np.float32), (P, 1))
    b1c = b1.reshape(P, 1).astype(np.float32)
    b2c = np.tile(b2.reshape(1, O), (P, 1)).astype(np.float32)

    dt_tab = mybir.dt.bfloat16
    dt_f32 = mybir.dt.bfloat16
    nc = _build_program(T, NBLK, NSH, NPAD, dt_tab, dt_f32)
    bf16 = mybir.dt.np(mybir.dt.bfloat16)

    in_maps = []
    for c in range(NCORES):
        lo, hi = c * T * NBLK, (c + 1) * T * NBLK
        in_maps.append({
            "xb": xb.astype(bf16),
            "esrc": np.ascontiguousarray(esrc_b[lo:hi].T),
            "edstl": np.ascontiguousarray(edstl_b[lo:hi].T).astype(bf16),
            "ecw": np.ascontiguousarray(ecw_b[lo:hi].T).astype(bf16),
            "w1": W1.astype(bf16),
            "b1": b1c,
            "w2": W2.astype(bf16),
            "b2": b2c,
            "iota": iota_np.astype(bf16),
        })

    res = run_bass_kernel_spmd(nc, in_maps, list(range(NCORES)))
    LAST_RESULT = res
    out_full = np.concatenate([res.results[c]["out"] for c in range(NCORES)], axis=0)
    return out_full[:N].astype(np.float32)
